# revision 8
# baseline (speedup 1.0000x reference)
"""Trainium2 Bass kernel for DeepProteinClassifier.

Contract: kernel(**inputs) takes the FULL unsharded inputs (see shapes
below) and returns the FULL [32, 10] float32 output.

Sharding: data-parallel over batch B=32 across 8 NeuronCores (4 samples
per core); all weights replicated.

Per-core device pipeline (all big matmuls bf16 with fp32 PSUM accum):
  for each of 4 samples:
    QT/KT [d,s] = W^T-chunks @ xT-chunks     (bias via ACT drain, Q scaled 1/sqrt(D))
    V [s,d|1]   = xT-chunks^T @ Wv-chunks    (bias via rank-1 matmul; ones col)
    ST [k,q]    = KT^T-chunks @ QT-chunks; ET = exp(ST + maskneg_k)  (mask via ACT bias)
    CTXu [q,d|r]= ET-chunks^T @ V-chunks     (r = softmax denom from ones col)
    H = CTXu/r + x                           (ACT scale-copy + DVE fused add+rowsum)
    LN-stats + masked-pool folded into a PE matvec: pooled = sum_s alpha_s * [H|mu]
        with alpha = mask/summask * rsqrt(var+eps)  (pooled - alpha.mu correction col)
  after 4 samples: pooled[4,961] -> subtract c col -> PE-transpose -> 4-layer MLP
    (transposed layout, biases+relu via ACT drains) -> out [10, 4]
Host only does layout prep (transpose/cast) and final [10,4]->[4,10] gather.
"""

import numpy as np
import ml_dtypes

B, S, D = 32, 1024, 960
NCORES = 8
BPC = B // NCORES  # 4 samples per core
PD = 120           # partition size for d-chunks (960 = 8*120)
NDC = 8            # number of d chunks
PT = 128           # partition size for s-tiles (1024 = 8*128)
NST = 8            # number of s tiles
LN_EPS = 1e-5
BF16 = ml_dtypes.bfloat16

_CACHE = {}


def _build_nc():
    import concourse.tile as tile
    from concourse import bacc, mybir

    f32 = mybir.dt.float32
    bf16 = mybir.dt.bfloat16
    Alu = mybir.AluOpType
    Act = mybir.ActivationFunctionType

    nc = bacc.Bacc("TRN2", target_bir_lowering=False, debug=False)

    # ---- DRAM parameters (per-core shard) ----
    xt_h = nc.declare_dram_parameter("xt", [BPC, D, S], bf16, isOutput=False)
    xn_h = nc.declare_dram_parameter("xn", [BPC, S, D], bf16, isOutput=False)
    xs_h = nc.declare_dram_parameter("xs", [BPC, PT, NST], f32, isOutput=False)
    mnp_h = nc.declare_dram_parameter("mnp", [BPC, PT, NST], f32, isOutput=False)
    mfs_h = nc.declare_dram_parameter("mfs", [BPC, PT, NST], f32, isOutput=False)
    wq_h = nc.declare_dram_parameter("wq", [D, D], bf16, isOutput=False)
    wk_h = nc.declare_dram_parameter("wk", [D, D], bf16, isOutput=False)
    wv_h = nc.declare_dram_parameter("wv", [D, D], bf16, isOutput=False)
    bq_h = nc.declare_dram_parameter("bq", [PD, NDC], f32, isOutput=False)
    bk_h = nc.declare_dram_parameter("bk", [PD, NDC], f32, isOutput=False)
    w1_h = nc.declare_dram_parameter("w1", [D, 512], bf16, isOutput=False)
    w2_h = nc.declare_dram_parameter("w2", [512, 256], bf16, isOutput=False)
    w3_h = nc.declare_dram_parameter("w3", [256, 128], bf16, isOutput=False)
    w4_h = nc.declare_dram_parameter("w4", [128, 10], bf16, isOutput=False)
    b1_h = nc.declare_dram_parameter("b1", [128, 4], f32, isOutput=False)
    b2_h = nc.declare_dram_parameter("b2", [128, 2], f32, isOutput=False)
    b3_h = nc.declare_dram_parameter("b3", [128, 1], f32, isOutput=False)
    b4_h = nc.declare_dram_parameter("b4", [10, 1], f32, isOutput=False)
    id4_h = nc.declare_dram_parameter("id4", [4, 4], f32, isOutput=False)
    out_h = nc.declare_dram_parameter("out", [10, BPC], f32, isOutput=True)

    with tile.TileContext(nc) as tc:
        with (
            tc.tile_pool(name="wpool", bufs=1) as wpool,
            tc.tile_pool(name="xpool", bufs=2) as xpool,
            tc.tile_pool(name="big", bufs=1) as big,
            tc.tile_pool(name="stats", bufs=2) as stats,
            tc.tile_pool(name="psum", bufs=7, space="PSUM") as psum,
        ):
            # ---- resident weights ----
            wq_sb = wpool.tile([PD, NDC, D], bf16)
            nc.sync.dma_start(wq_sb[:], wq_h[:].rearrange("(c p) n -> p c n", p=PD))
            wk_sb = wpool.tile([PD, NDC, D], bf16)
            nc.sync.dma_start(wk_sb[:], wk_h[:].rearrange("(c p) n -> p c n", p=PD))
            wv_sb = wpool.tile([PD, NDC, D], bf16)
            nc.sync.dma_start(wv_sb[:], wv_h[:].rearrange("(c p) n -> p c n", p=PD))
            w1_sb = wpool.tile([PD, NDC, 512], bf16)
            nc.sync.dma_start(w1_sb[:], w1_h[:].rearrange("(c p) n -> p c n", p=PD))
            w2_sb = wpool.tile([128, 4, 256], bf16)
            nc.sync.dma_start(w2_sb[:], w2_h[:].rearrange("(c p) n -> p c n", p=128))
            w3_sb = wpool.tile([128, 2, 128], bf16)
            nc.sync.dma_start(w3_sb[:], w3_h[:].rearrange("(c p) n -> p c n", p=128))
            w4_sb = wpool.tile([128, 10], bf16)
            nc.sync.dma_start(w4_sb[:], w4_h[:])
            bq_sb = wpool.tile([PD, NDC], f32)
            nc.sync.dma_start(bq_sb[:], bq_h[:])
            bk_sb = wpool.tile([PD, NDC], f32)
            nc.sync.dma_start(bk_sb[:], bk_h[:])
            b1_sb = wpool.tile([128, 4], f32)
            nc.sync.dma_start(b1_sb[:], b1_h[:])
            b2_sb = wpool.tile([128, 2], f32)
            nc.sync.dma_start(b2_sb[:], b2_h[:])
            b3_sb = wpool.tile([128, 1], f32)
            nc.sync.dma_start(b3_sb[:], b3_h[:])
            b4_sb = wpool.tile([10, 1], f32)
            nc.sync.dma_start(b4_sb[:], b4_h[:])
            id4_sb = wpool.tile([4, 4], f32)
            nc.sync.dma_start(id4_sb[:], id4_h[:])
            pooled_all = wpool.tile([BPC, D + 1], f32)

            for j in range(BPC):
                xt_sb = xpool.tile([PD, NDC, S], bf16, tag="xt", name=f"xt{j}")
                nc.sync.dma_start(
                    xt_sb[:], xt_h[j].rearrange("(c p) s -> p c s", p=PD)
                )
                xn_sb = xpool.tile([PT, NST, D], bf16, tag="xn", name=f"xn{j}", bufs=1)
                nc.sync.dma_start(
                    xn_sb[:], xn_h[j].rearrange("(t p) d -> p t d", p=PT)
                )
                xs_sb = stats.tile([PT, NST], f32, tag="xs", name=f"xs{j}")
                nc.sync.dma_start(xs_sb[:], xs_h[j])
                mnp_sb = stats.tile([PT, NST], f32, tag="mnp", name=f"mnp{j}")
                nc.sync.dma_start(mnp_sb[:], mnp_h[j])
                mfs_sb = stats.tile([PT, NST], f32, tag="mfs", name=f"mfs{j}")
                nc.sync.dma_start(mfs_sb[:], mfs_h[j])

                # ---- Q^T, K^T projections: [d_out on partitions, s free] ----
                QT = big.tile([PD, NDC, S], bf16, tag="QT", name=f"QT{j}")
                KT = big.tile([PD, NDC, S], bf16, tag="KT", name=f"KT{j}")
                for w_sb, b_sb, OUT in ((wq_sb, bq_sb, QT), (wk_sb, bk_sb, KT)):
                    for t in range(NDC):
                        for sc in range(2):
                            ps = psum.tile([128, 512], f32, tag="mm", name="psq")
                            for c in range(NDC):
                                nc.tensor.matmul(
                                    ps[:PD, :],
                                    lhsT=w_sb[:, c, t * PD : (t + 1) * PD],
                                    rhs=xt_sb[:, c, sc * 512 : (sc + 1) * 512],
                                    start=(c == 0),
                                    stop=(c == NDC - 1),
                                )
                            nc.scalar.activation(
                                OUT[:, t, sc * 512 : (sc + 1) * 512],
                                ps[:PD, :],
                                Act.Identity,
                                bias=b_sb[:, t : t + 1],
                            )

                # ---- V in natural layout [s on partitions, d|1 free] ----
                V = big.tile([PT, NST, D + 1], bf16, tag="V", name=f"V{j}")
                nc.vector.memset(V[:, :, D : D + 1], 1.0)
                for st in range(NST):
                    for lo, hi in ((0, 512), (512, D)):
                        ps = psum.tile([128, 512], f32, tag="mm", name="psv")
                        for c in range(NDC):
                            nc.tensor.matmul(
                                ps[:, : hi - lo],
                                lhsT=xt_sb[:, c, st * PT : (st + 1) * PT],
                                rhs=wv_sb[:, c, lo:hi],
                                start=(c == 0),
                                stop=(c == NDC - 1),
                            )
                        nc.scalar.activation(
                            V[:, st, lo:hi], ps[:, : hi - lo], Act.Copy
                        )

                # ---- transposed scores + exp (mask as per-partition bias) ----
                ET = big.tile([PT, NST, S], bf16, tag="ET", name=f"ET{j}")
                for kt in range(NST):
                    for qc in range(2):
                        ps = psum.tile([128, 512], f32, tag="mm", name="pss")
                        for t in range(NDC):
                            nc.tensor.matmul(
                                ps[:],
                                lhsT=KT[:, t, kt * PT : (kt + 1) * PT],
                                rhs=QT[:, t, qc * 512 : (qc + 1) * 512],
                                start=(t == 0),
                                stop=(t == NDC - 1),
                            )
                        nc.scalar.activation(
                            ET[:, kt, qc * 512 : (qc + 1) * 512],
                            ps[:],
                            Act.Exp,
                            bias=mnp_sb[:, kt : kt + 1],
                        )

                # ---- context + residual + LN stats ----
                H = big.tile([PT, NST, D + 1], bf16, tag="H", name=f"H{j}")
                CS0 = stats.tile([PT, NST], f32, tag="CS0", name=f"CS0{j}")
                CS1 = stats.tile([PT, NST], f32, tag="CS1", name=f"CS1{j}")
                SQ = stats.tile([PT, NST], f32, tag="SQ", name=f"SQ{j}")
                recips = stats.tile([PT, NST], f32, tag="recips", name=f"rc{j}")
                for qt in range(NST):
                    ps0 = psum.tile([128, 512], f32, tag="mm", name="psc0")
                    ps1 = psum.tile([128, 512], f32, tag="mm", name="psc1")
                    for kc in range(NST):
                        nc.tensor.matmul(
                            ps0[:],
                            lhsT=ET[:, kc, qt * PT : (qt + 1) * PT],
                            rhs=V[:, kc, 0:512],
                            start=(kc == 0),
                            stop=(kc == NST - 1),
                        )
                    for kc in range(NST):
                        nc.tensor.matmul(
                            ps1[:, : D + 1 - 512],
                            lhsT=ET[:, kc, qt * PT : (qt + 1) * PT],
                            rhs=V[:, kc, 512 : D + 1],
                            start=(kc == 0),
                            stop=(kc == NST - 1),
                        )
                    # r (softmax denom) is the last accumulated column
                    nc.vector.reciprocal(
                        recips[:, qt : qt + 1], ps1[:, D - 512 : D + 1 - 512]
                    )
                    # ctx = psum/r, with running row-sums via ACT accum
                    ctx0 = stats.tile([PT, 512], bf16, tag="ctx0", name=f"c0_{j}_{qt}")
                    ctx1 = stats.tile([PT, D - 512], bf16, tag="ctx1", name=f"c1_{j}_{qt}")
                    nc.scalar.activation(
                        ctx0[:],
                        ps0[:],
                        Act.Copy,
                        scale=recips[:, qt : qt + 1],
                        accum_out=CS0[:, qt : qt + 1],
                    )
                    nc.scalar.activation(
                        ctx1[:],
                        ps1[:, 0 : D - 512],
                        Act.Copy,
                        scale=recips[:, qt : qt + 1],
                        accum_out=CS1[:, qt : qt + 1],
                    )
                    # H = ctx + (x + bv)
                    nc.vector.tensor_add(
                        H[:, qt, 0:512], ctx0[:], xn_sb[:, qt, 0:512]
                    )
                    nc.vector.tensor_add(
                        H[:, qt, 512:D], ctx1[:], xn_sb[:, qt, 512:D]
                    )
                    # sum of squares via ACT Square accumulate
                    scratch = stats.tile(
                        [PT, D], bf16, tag="scr", name=f"scr{j}_{qt}", bufs=1
                    )
                    nc.scalar.activation(
                        scratch[:],
                        H[:, qt, 0:D],
                        Act.Square,
                        accum_out=SQ[:, qt : qt + 1],
                    )

                # ---- LN scalars -> alpha; mu column ----
                # mu = (sum(ctx) + sum(x + bv)) / D
                MU = stats.tile([PT, NST], f32, tag="MU", name=f"MU{j}")
                nc.vector.tensor_add(MU[:], CS0[:], CS1[:])
                nc.vector.tensor_add(MU[:], MU[:], xs_sb[:])
                nc.vector.tensor_scalar_mul(MU[:], MU[:], 1.0 / D)
                nc.vector.tensor_copy(H[:, :, D : D + 1], MU[:, :, None])
                VAR = stats.tile([PT, NST], f32, tag="VAR", name=f"VAR{j}")
                T2 = stats.tile([PT, NST], f32, tag="T2", name=f"T2{j}")
                nc.vector.tensor_scalar_mul(T2[:], SQ[:], 1.0 / D)
                nc.vector.tensor_tensor(VAR[:], MU[:], MU[:], Alu.mult)
                nc.vector.tensor_sub(VAR[:], T2[:], VAR[:])
                nc.vector.tensor_scalar_add(VAR[:], VAR[:], LN_EPS)
                # rs = exp(-0.5 * ln(var+eps))
                nc.scalar.activation(VAR[:], VAR[:], Act.Ln)
                RS = stats.tile([PT, NST], f32, tag="RS", name=f"RS{j}")
                nc.scalar.activation(RS[:], VAR[:], Act.Exp, scale=-0.5)
                AL = stats.tile([PT, NST], bf16, tag="AL", name=f"AL{j}")
                nc.vector.tensor_tensor(AL[:], mfs_sb[:], RS[:], Alu.mult)

                # ---- masked-mean pool as PE matvec: pooled = sum_s AL * [H|mu] ----
                pp0 = psum.tile([128, 512], f32, tag="mm", name="pp0")
                pp1 = psum.tile([128, 512], f32, tag="mm", name="pp1")
                for c in range(NST):
                    nc.tensor.matmul(
                        pp0[:1, :],
                        lhsT=AL[:, c : c + 1],
                        rhs=H[:, c, 0:512],
                        start=(c == 0),
                        stop=(c == NST - 1),
                    )
                for c in range(NST):
                    nc.tensor.matmul(
                        pp1[:1, : D + 1 - 512],
                        lhsT=AL[:, c : c + 1],
                        rhs=H[:, c, 512 : D + 1],
                        start=(c == 0),
                        stop=(c == NST - 1),
                    )
                prow = stats.tile([1, D + 1], f32, tag="prow", name=f"prow{j}", bufs=1)
                nc.scalar.activation(prow[:, 0:512], pp0[:1, :], Act.Copy)
                nc.scalar.activation(
                    prow[:, 512 : D + 1], pp1[:1, : D + 1 - 512], Act.Copy
                )
                nc.sync.dma_start(pooled_all[j : j + 1, :], prow[:])

            # ---- pooled correction + transpose ----
            pooled_f = stats.tile([BPC, D], f32, tag="pf")
            nc.vector.tensor_scalar(
                pooled_f[:],
                pooled_all[:, 0:D],
                pooled_all[:, D : D + 1],
                None,
                Alu.subtract,
                Alu.bypass,
            )
            pooledT = stats.tile([PD, NDC, BPC], bf16, tag="pT")
            for c in range(NDC):
                pst = psum.tile([128, 512], f32, tag="mm", name=f"pst{c}")
                nc.tensor.transpose(
                    pst[:PD, :BPC],
                    pooled_f[:, c * PD : (c + 1) * PD],
                    id4_sb[:],
                )
                nc.scalar.activation(pooledT[:, c, :], pst[:PD, :BPC], Act.Copy)

            # ---- MLP in transposed layout ----
            h1T = stats.tile([128, 4, BPC], bf16, tag="h1T")
            for m in range(4):
                ps = psum.tile([128, 512], f32, tag="mm", name=f"psm1{m}")
                for c in range(NDC):
                    nc.tensor.matmul(
                        ps[:, :BPC],
                        lhsT=w1_sb[:, c, m * 128 : (m + 1) * 128],
                        rhs=pooledT[:, c, :],
                        start=(c == 0),
                        stop=(c == NDC - 1),
                    )
                nc.scalar.activation(
                    h1T[:, m, :], ps[:, :BPC], Act.Relu, bias=b1_sb[:, m : m + 1]
                )
            h2T = stats.tile([128, 2, BPC], bf16, tag="h2T")
            for m in range(2):
                ps = psum.tile([128, 512], f32, tag="mm", name=f"psm2{m}")
                for c in range(4):
                    nc.tensor.matmul(
                        ps[:, :BPC],
                        lhsT=w2_sb[:, c, m * 128 : (m + 1) * 128],
                        rhs=h1T[:, c, :],
                        start=(c == 0),
                        stop=(c == 3),
                    )
                nc.scalar.activation(
                    h2T[:, m, :], ps[:, :BPC], Act.Relu, bias=b2_sb[:, m : m + 1]
                )
            h3T = stats.tile([128, 1, BPC], bf16, tag="h3T")
            ps = psum.tile([128, 512], f32, tag="mm", name="psm3")
            for c in range(2):
                nc.tensor.matmul(
                    ps[:, :BPC],
                    lhsT=w3_sb[:, c, :],
                    rhs=h2T[:, c, :],
                    start=(c == 0),
                    stop=(c == 1),
                )
            nc.scalar.activation(
                h3T[:, 0, :], ps[:, :BPC], Act.Relu, bias=b3_sb[:, 0:1]
            )
            ps4 = psum.tile([128, 512], f32, tag="mm", name="psm4")
            nc.tensor.matmul(
                ps4[:10, :BPC], lhsT=w4_sb[:, :], rhs=h3T[:, 0, :],
                start=True, stop=True,
            )
            osb = stats.tile([10, BPC], f32, tag="osb")
            nc.scalar.activation(osb[:], ps4[:10, :BPC], Act.Identity, bias=b4_sb[:])
            nc.sync.dma_start(out_h[:], osb[:])

    nc.compile()
    return nc


def _get_nc():
    if "nc" not in _CACHE:
        _CACHE["nc"] = _build_nc()
    return _CACHE["nc"]


def host_prep(inputs):
    """Build the 8 per-core in_maps from the full inputs."""
    x = np.asarray(inputs["x"], np.float32)
    mask = np.asarray(inputs["mask"])
    Wq, bq = np.asarray(inputs["Wq"], np.float32), np.asarray(inputs["bq"], np.float32)
    Wk, bk = np.asarray(inputs["Wk"], np.float32), np.asarray(inputs["bk"], np.float32)
    Wv, bv = np.asarray(inputs["Wv"], np.float32), np.asarray(inputs["bv"], np.float32)
    ln_g, ln_b = np.asarray(inputs["ln_g"], np.float32), np.asarray(inputs["ln_b"], np.float32)
    W1, b1 = np.asarray(inputs["W1"], np.float32), np.asarray(inputs["b1"], np.float32)
    W2, b2 = np.asarray(inputs["W2"], np.float32), np.asarray(inputs["b2"], np.float32)
    W3, b3 = np.asarray(inputs["W3"], np.float32), np.asarray(inputs["b3"], np.float32)
    W4, b4 = np.asarray(inputs["W4"], np.float32), np.asarray(inputs["b4"], np.float32)

    isq = 1.0 / np.sqrt(np.float32(D))
    wq = np.ascontiguousarray((Wq.T * isq)).astype(BF16)
    wk = np.ascontiguousarray(Wk.T).astype(BF16)
    wv = np.ascontiguousarray(Wv.T).astype(BF16)
    bqp = np.ascontiguousarray((bq * isq).reshape(NDC, PD).T).astype(np.float32)
    bkp = np.ascontiguousarray(bk.reshape(NDC, PD).T).astype(np.float32)

    W1e = W1 * ln_g[None, :]
    b1e = b1 + W1 @ ln_b
    w1 = np.ascontiguousarray(W1e.T).astype(BF16)
    b1p = np.ascontiguousarray(b1e.reshape(4, 128).T).astype(np.float32)
    w2 = np.ascontiguousarray(W2.T).astype(BF16)
    b2p = np.ascontiguousarray(b2.reshape(2, 128).T).astype(np.float32)
    w3 = np.ascontiguousarray(W3.T).astype(BF16)
    b3p = np.ascontiguousarray(b3.reshape(1, 128).T).astype(np.float32)
    w4 = np.ascontiguousarray(W4.T).astype(BF16)
    b4p = np.ascontiguousarray(b4.reshape(10, 1)).astype(np.float32)
    id4 = np.eye(4, dtype=np.float32)

    mf = mask.astype(np.float32)                       # [B, S]
    msum = np.maximum(mf.sum(axis=1, keepdims=True), 1e-9)
    mfs_full = mf / msum                               # [B, S]
    mneg_full = np.where(mf == 0, np.float32(-1e9), np.float32(0.0))

    shared = dict(
        wq=wq, wk=wk, wv=wv, bq=bqp, bk=bkp,
        w1=w1, w2=w2, w3=w3, w4=w4,
        b1=b1p, b2=b2p, b3=b3p, b4=b4p, id4=id4,
    )
    in_maps = []
    for core in range(NCORES):
        sl = slice(core * BPC, (core + 1) * BPC)
        xc = x[sl]                                      # [4, S, D]
        xt = np.ascontiguousarray(xc.transpose(0, 2, 1)).astype(BF16)
        xnf = xc + bv[None, None, :]                    # fold V bias into residual
        xn = np.ascontiguousarray(xnf).astype(BF16)
        xs = np.ascontiguousarray(
            xn.astype(np.float32).sum(axis=2).reshape(BPC, NST, PT).transpose(0, 2, 1)
        ).astype(np.float32)
        mnp = np.ascontiguousarray(
            mneg_full[sl].reshape(BPC, NST, PT).transpose(0, 2, 1)
        ).astype(np.float32)
        mfs = np.ascontiguousarray(
            mfs_full[sl].reshape(BPC, NST, PT).transpose(0, 2, 1)
        ).astype(np.float32)
        m = dict(shared)
        m.update(xt=xt, xn=xn, xs=xs, mnp=mnp, mfs=mfs)
        in_maps.append(m)
    return in_maps


def assemble(results):
    """results: list of 8 dicts with 'out' [10, BPC] -> [32, 10] f32."""
    return np.concatenate(
        [np.asarray(r["out"], np.float32).T for r in results], axis=0
    )


def kernel(**inputs):
    from concourse.bass_utils import run_bass_kernel_spmd

    nc = _get_nc()
    in_maps = host_prep(inputs)
    res = run_bass_kernel_spmd(nc, in_maps, core_ids=list(range(NCORES)))
    return assemble(res.results)
